# revision 1
# baseline (speedup 1.0000x reference)
"""OmicsEmbeddingLayer Trainium2 kernel.

Computation (per the reference):
    feat = emb[gene_idx]                  # [L, H] gather
    h    = x @ feat                       # [B, H]
    h2   = relu(h @ W1 + b1)              # [B, H]
    out  = LayerNorm(h2) * gamma + beta   # [B, H]

Sharding: data-parallel over cells (B) across 8 cores; emb/W1/norm params
replicated. Inside each core the big matmul contracts over genes (L), so x is
fed pre-transposed (x^T sliced per core, prepared host-side during sharding)
which puts the contraction dim on SBUF partitions with no on-device
transposes anywhere in the pipeline:

    MM1: h^T[jh, m] = sum_k feat[k, jh] * x^T[k, m]     (feat stationary)
    MM2: h2[m, :]   = sum_j h^T[j,:]^T W1[j,:] (+ b1)   (h^T stationary)

MM2's output lands in natural [cells, H] layout, so the ReLU + LayerNorm
epilogue reduces along the free axis and the output DMA is dense.

x is read from HBM in full fp32 and cast to fp16 on-chip (DVE, hidden under
the DMA); fp16 keeps a 10-bit mantissa (tf32-class accuracy for this
computation) while running the PE at 1 cycle/row with overlapped weight
loads (fp32r would force self-loading matmuls: +204ns serialized LDWEIGHTS
per matmul).

Cells are processed in two half-passes of 1024 so the first half's
MM2/LayerNorm epilogue overlaps the second half's matmul stream.
"""

import sys

if "/opt/trn_rl_repo" not in sys.path:
    sys.path.insert(0, "/opt/trn_rl_repo")

import numpy as np

B, L, G, H = 16384, 4096, 30000, 256
N_CORES = 8
BS = B // N_CORES          # 2048 cells per core
HALF = 1024                # cells per half-pass
KC = L // 128              # 32 contraction chunks of 128 genes
KSLAB = 4                  # k-chunks per x DMA slab (2 MB each)
NSLAB = KC // KSLAB        # 8 slabs per half
GATHER_ROWS = [512, 512, 1024, 1024, 1024]             # small-first schedule
XSLAB_K0 = [1, 1] + [2] * 15                           # first-half k-slab sizes
XSLAB_K1 = [2] * 16                                    # second-half k-slab sizes
EPS = 1e-5

_CACHE: dict = {}


def _build_nc(with_b1: bool, with_gamma: bool, with_beta: bool):
    import concourse.bacc as bacc
    import concourse.mybir as mybir
    import concourse.tile as tile
    from concourse import library_config

    f32 = mybir.dt.float32
    f16 = mybir.dt.float16
    AF = mybir.ActivationFunctionType
    OP = mybir.AluOpType

    nc = bacc.Bacc("TRN2", num_swdge_queues=2, dynamic_dma_scratch_size=32768)
    xt = nc.dram_tensor("xt", [L, BS], f32, kind="ExternalInput")
    emb = nc.dram_tensor("emb", [G, H], f32, kind="ExternalInput")
    idx = nc.dram_tensor("idx", [128, L // 16], mybir.dt.int16, kind="ExternalInput")
    w1 = nc.dram_tensor("w1", [H, H], f32, kind="ExternalInput")
    b1 = nc.dram_tensor("b1", [1, H], f32, kind="ExternalInput")
    gamma = nc.dram_tensor("gamma", [1, H], f32, kind="ExternalInput")
    beta = nc.dram_tensor("beta", [1, H], f32, kind="ExternalInput")
    out = nc.dram_tensor("out", [BS, H], f32, kind="ExternalOutput")

    with tile.TileContext(nc) as tc:
        with (
            tc.tile_pool(name="consts", bufs=1) as consts,
            tc.tile_pool(name="gpool", bufs=3) as gpool,
            tc.tile_pool(name="x32pool", bufs=8) as x32pool,
            tc.tile_pool(name="x16pool", bufs=12) as x16pool,
            tc.tile_pool(name="hpool", bufs=2) as hpool,
            tc.tile_pool(name="epool", bufs=4) as epool,
            tc.tile_pool(name="opool", bufs=2) as opool,
            tc.tile_pool(name="accp", bufs=6, space="PSUM") as accp,
            tc.tile_pool(name="ps2", bufs=2, space="PSUM") as ps2,
        ):
            # dma_gather lives in the mlp gpsimd library
            nc.gpsimd.load_library(library_config.mlp)

            # ---- constants / small inputs ----
            idx_sb = consts.tile([128, L // 16], mybir.dt.int16)
            nc.sync.dma_start(out=idx_sb[:], in_=idx[:, :])

            # feat[k*128+p, h] gathered with a small-first call schedule so
            # the first k-chunks land quickly; each call has its own staging
            # and fp16 tile so matmuls depend only on their own gather call.
            feat_chunk = []  # k-chunk -> (tile, local offset)
            row0 = 0
            for s, rows in enumerate(GATHER_ROWS):
                kpc = rows // 128
                fstage = gpool.tile([128, kpc, H], f32, tag="fstage")
                nc.gpsimd.dma_gather(
                    fstage[:],
                    emb[:, :],
                    idx_sb[:, row0 // 16 : (row0 + rows) // 16],
                    rows,
                    rows,
                    H,
                    queue_num=s % 2,
                )
                f16t = consts.tile([128, kpc, H], f16, tag=f"feat16_{s}")
                nc.scalar.copy(out=f16t[:], in_=fstage[:])
                for kk in range(kpc):
                    feat_chunk.append((f16t, kk))
                row0 += rows

            # W1 chunks: w116[p, j, :] = W1[j*128+p, :]
            w1_32 = consts.tile([128, 2, H], f32)
            nc.scalar.dma_start(out=w1_32[:], in_=w1.rearrange("(j p) h -> p j h", p=128))
            w116 = consts.tile([128, 2, H], f16)
            nc.scalar.copy(out=w116[:], in_=w1_32[:])

            if with_b1:
                b1_sb = consts.tile([128, H], f32)
                nc.gpsimd.dma_start(out=b1_sb[:], in_=b1[:, :].to_broadcast([128, H]))
            if with_gamma:
                gamma_sb = consts.tile([128, H], f32)
                nc.gpsimd.dma_start(out=gamma_sb[:], in_=gamma[:, :].to_broadcast([128, H]))
            if with_beta:
                beta_sb = consts.tile([128, H], f32)
                nc.gpsimd.dma_start(out=beta_sb[:], in_=beta[:, :].to_broadcast([128, H]))
            eps_sb = consts.tile([128, 1], f32)
            nc.vector.memset(eps_sb[:], EPS)

            # xt rows (k*128+p) -> partition p, k-chunk k
            xt_r = xt.rearrange("(k p) m -> p k m", p=128)   # [128, KC, BS]
            out_r = out.rearrange("(g p) h -> g p h", p=128)  # [16, 128, 256]

            for hh in range(2):  # half-pass over cells
                c0 = hh * HALF
                slab_ks = XSLAB_K0 if hh == 0 else XSLAB_K1
                x16_chunk = []  # k-chunk -> (tile, local offset)
                k0 = 0
                for ks in slab_ks:
                    x32 = x32pool.tile([128, ks, HALF], f32, tag="x32")
                    nc.sync.dma_start(
                        out=x32[:],
                        in_=xt_r[:, k0 : k0 + ks, c0 : c0 + HALF],
                    )
                    x16 = x16pool.tile([128, ks, HALF], f16, tag="x16")
                    nc.vector.tensor_copy(out=x16[:], in_=x32[:])
                    for kk in range(ks):
                        x16_chunk.append((x16, kk))
                    k0 += ks

                # MM1: 4 accumulators (2 H-halves x 2 cell-512-groups)
                accs = {}
                for j in range(2):
                    for m in range(2):
                        acc_t = accp.tile([128, 512], f32, tag="acc")
                        accs[j, m] = acc_t
                for k in range(KC):
                    xt16, kl = x16_chunk[k]
                    for j in range(2):
                        for m in range(2):
                            nc.tensor.matmul(
                                out=accs[j, m][:],
                                lhsT=feat_chunk[k][0][:, feat_chunk[k][1], j * 128 : (j + 1) * 128],
                                rhs=xt16[:, kl, m * 512 : (m + 1) * 512],
                                start=(k == 0),
                                stop=(k == KC - 1),
                            )

                hT = hpool.tile([128, 2, HALF], f16, tag="hT")
                for j in range(2):
                    for m in range(2):
                        nc.scalar.copy(
                            out=hT[:, j, m * 512 : (m + 1) * 512], in_=accs[j, m][:]
                        )

                # MM2 + ReLU + LayerNorm per 128-cell subtile
                groups = [4, 2, 2] if hh == 1 else [4, 4]
                tbase = 0
                for g, gsz in enumerate(groups):
                    out_sb = opool.tile([128, 4, H], f32, tag="out_sb")
                    for t4 in range(gsz):
                        t = tbase + t4  # subtile within the half
                        p2 = ps2.tile([128, H], f32, tag="ps2")
                        for j in range(2):
                            nc.tensor.matmul(
                                out=p2[:],
                                lhsT=hT[:, j, t * 128 : (t + 1) * 128],
                                rhs=w116[:, j, :],
                                start=(j == 0),
                                stop=(j == 1),
                            )
                        h2 = epool.tile([128, H], f32, tag="h2")
                        if with_b1:
                            nc.vector.tensor_tensor(
                                out=h2[:], in0=p2[:], in1=b1_sb[:], op=OP.add
                            )
                            nc.scalar.activation(out=h2[:], in_=h2[:], func=AF.Relu)
                        else:
                            nc.scalar.activation(out=h2[:], in_=p2[:], func=AF.Relu)
                        stats = epool.tile([128, 6], f32, tag="stats")
                        nc.vector.bn_stats(out=stats[:], in_=h2[:])
                        mv = epool.tile([128, 2], f32, tag="mv")
                        nc.vector.bn_aggr(out=mv[:], in_=stats[:])
                        rstd = epool.tile([128, 1], f32, tag="rstd")
                        nc.scalar.activation(
                            out=rstd[:], in_=mv[:, 1:2], func=AF.Sqrt,
                            bias=eps_sb[:], scale=1.0,
                        )
                        nc.vector.reciprocal(out=rstd[:], in_=rstd[:])
                        y_out = out_sb[:, t4, :]
                        nc.vector.tensor_scalar(
                            out=y_out,
                            in0=h2[:],
                            scalar1=mv[:, 0:1],
                            scalar2=rstd[:],
                            op0=OP.subtract,
                            op1=OP.mult,
                        )
                        if with_gamma:
                            nc.vector.tensor_mul(y_out, y_out, gamma_sb[:])
                        if with_beta:
                            nc.vector.tensor_add(y_out, y_out, beta_sb[:])
                    gg = hh * 8 + tbase  # global 128-row group index
                    nc.scalar.dma_start(
                        out=out_r[gg : gg + gsz].transpose([1, 0, 2]),
                        in_=out_sb[:, :gsz, :],
                    )
                    tbase += gsz

    nc.compile()
    return nc


def _get_nc(with_b1, with_gamma, with_beta):
    key = ("nc", with_b1, with_gamma, with_beta)
    if key not in _CACHE:
        _CACHE[key] = _build_nc(with_b1, with_gamma, with_beta)
    return _CACHE[key]


def _prep(x, emb, W1, b1, gamma, beta, gene_idx):
    x = np.asarray(x, dtype=np.float32)
    emb = np.ascontiguousarray(np.asarray(emb, dtype=np.float32))
    W1 = np.ascontiguousarray(np.asarray(W1, dtype=np.float32))
    b1 = np.asarray(b1, dtype=np.float32).reshape(1, H)
    gamma = np.asarray(gamma, dtype=np.float32).reshape(1, H)
    beta = np.asarray(beta, dtype=np.float32).reshape(1, H)
    gi = np.asarray(gene_idx).astype(np.int64)
    assert gi.shape == (L,) and gi.min() >= 0 and gi.max() < G

    flags = (
        bool(np.any(b1 != 0.0)),
        bool(np.any(gamma != 1.0)),
        bool(np.any(beta != 0.0)),
    )

    # dma_gather index layout: position j -> partition j%16, column j//16,
    # replicated across the 8 gpsimd cores (partition groups of 16).
    wrap = gi.astype(np.int16).reshape(L // 16, 16).T      # [16, L//16]
    idx_arr = np.ascontiguousarray(np.tile(wrap, (8, 1)))  # [128, L//16]

    in_maps = []
    for c in range(N_CORES):
        xt_c = np.ascontiguousarray(x[c * BS : (c + 1) * BS, :].T)  # [L, BS]
        in_maps.append(
            {
                "xt": xt_c,
                "emb": emb,
                "idx": idx_arr,
                "w1": W1,
                "b1": b1,
                "gamma": gamma,
                "beta": beta,
            }
        )
    return in_maps, flags


def _ensure_ntff_hook():
    """Register the axon NTFF profile hook if the image's antenv lacks it."""
    import types

    try:
        import antenv.axon_hooks  # noqa: F401

        return
    except ImportError:
        pass
    try:
        from trn_agent_boot.trn_boot import _ntff_profile_via_ctypes

        hook = _ntff_profile_via_ctypes("/opt/axon/libaxon_pjrt.so")
    except Exception:
        return
    mod = types.ModuleType("antenv.axon_hooks")
    mod._hook = hook

    def set_axon_ntff_profile_hook(h):
        mod._hook = h

    def get_axon_ntff_profile_hook():
        return mod._hook

    mod.set_axon_ntff_profile_hook = set_axon_ntff_profile_hook
    mod.get_axon_ntff_profile_hook = get_axon_ntff_profile_hook
    sys.modules["antenv.axon_hooks"] = mod
    import antenv

    antenv.axon_hooks = mod


def _run(in_maps, flags, trace=False):
    from concourse.bass_utils import run_bass_kernel_spmd

    if trace:
        _ensure_ntff_hook()
    nc = _get_nc(*flags)
    return run_bass_kernel_spmd(
        nc, in_maps, core_ids=list(range(N_CORES)), trace=trace
    )


def kernel(x, emb, W1, b1, gamma, beta, gene_idx):
    in_maps, flags = _prep(x, emb, W1, b1, gamma, beta, gene_idx)
    res = _run(in_maps, flags)
    return np.concatenate([res.results[c]["out"] for c in range(N_CORES)], axis=0)


def kernel_traced(x, emb, W1, b1, gamma, beta, gene_idx):
    """Like kernel() but returns (output, BassKernelResults) with profiling."""
    in_maps, flags = _prep(x, emb, W1, b1, gamma, beta, gene_idx)
    res = _run(in_maps, flags, trace=True)
    out = np.concatenate([res.results[c]["out"] for c in range(N_CORES)], axis=0)
    return out, res

